# revision 1
# baseline (speedup 1.0000x reference)
"""DatasetTopK Trainium2 kernel.

Problem: query_embeddings [1024, 64] f32, candidates [1048576, 64] f32
-> per-query top-100 scores (sorted desc), scores = Q @ C^T.

Strategy (8 NeuronCores, candidates sharded 131072/core):
  - Host: transpose + pack each core's candidate shard into [128, 65536]
    (superblocks of 1024 candidates split across the two 64-partition
    halves, enabling 2-way row-tiled K=64 matmuls on the PE array).
  - Device: bf16 matmuls (full PE rate, ~0.1 abs err) -> PSUM
    [128q, 2048c] 4-bank groups; the PSUM scan is split between the only
    two engines with PSUM access, ratio-balanced to their throughputs:
      * DVE max8 (top-8 of each 2048-block) on ~50% of groups
      * ACT relu(s - t_q) + accum (block screening) on the rest; host
        rescores flagged blocks exactly.
  - Host: thresholds t_q from an exact 1/64 sample pass; final exact
    top-100 merge of DVE survivors + rescored ACT candidates.
"""

import numpy as np

import concourse.bass as bass
import concourse.mybir as mybir
from concourse.tile import TileContext
from concourse.bass_utils import run_bass_kernel_spmd

F32 = mybir.dt.float32
F32R = mybir.dt.float32r
BF16 = mybir.dt.bfloat16

_NCORES = 8
_NQ = 1024
_D = 64
_NCAND = 1048576
_SHARD = _NCAND // _NCORES  # 131072
_GRP = 2048  # candidates per scan tile = 4 PSUM banks
_NGRP = _SHARD // _GRP  # 64 groups per core
_SGG = 4  # groups per supergroup (DMA chunk: 8192 cands = 2 MiB)
_NSG = _NGRP // _SGG  # 16 supergroups
_NQT = 8  # query tiles of 128
_K = 100

_DVE_FRAC = 0.50  # fraction of (sg,qt,g) scan tiles handled by DVE max8
_SUM_EPS = 0.01  # ACT screen sum > eps -> host rescore
_M_SAMPLE = 6  # threshold = m-th best of the 1/64 sample
_T_MARGIN = 0.10

TRACE = False  # set by test harness for profiling runs

_ctr = [0]


def _split_sync_waits(nc, max_waits=1):
    """Workaround for walrus 'Too many sync wait commands': move excess
    per-instruction sync-waits onto preceding same-engine NOPs."""
    for f in nc.m.functions:
        for b in f.blocks:
            new_insts = []
            changed = False
            for ins in b.instructions:
                si = ins.sync_info
                if si is not None and len(si.on_wait) > max_waits:
                    waits = list(si.on_wait)
                    head, rest = waits[: -max_waits], waits[-max_waits:]
                    for i in range(0, len(head), max_waits):
                        _ctr[0] += 1
                        nop = mybir.InstNoOp(
                            name=f"I-waitsplit-{_ctr[0]}",
                            engine=ins.engine,
                            sync_info=mybir.SyncInfo(
                                on_wait=head[i : i + max_waits], on_update=[]
                            ),
                        )
                        nc.register_instruction(nop, overwrite=True)
                        new_insts.append(nop)
                        changed = True
                    ins.sync_info = mybir.SyncInfo(
                        on_wait=rest, on_update=list(si.on_update)
                    )
                new_insts.append(ins)
            if changed:
                b.instructions = new_insts
    return nc


def _is_dve(j):
    """Engine for the j-th scan tile in execution order (Bresenham mix)."""
    return int((j + 1) * _DVE_FRAC) != int(j * _DVE_FRAC)


def _build(nsg=_NSG, sgg=_SGG):
    ngrp = nsg * sgg
    shard = ngrp * _GRP
    nc = bass.Bass()
    q = nc.dram_tensor("q", [128, _NQ], BF16, kind="ExternalInput")
    cand = nc.dram_tensor("cand", [128, shard // 2], BF16, kind="ExternalInput")
    tq = nc.dram_tensor("tq", [128, _NQT], F32, kind="ExternalInput")
    out = nc.dram_tensor("out", [128, _NQT * ngrp * 8], F32, kind="ExternalOutput")
    sums = nc.dram_tensor("sums", [128, _NQT * ngrp], F32, kind="ExternalOutput")

    with TileContext(nc) as tc:
        with (
            tc.tile_pool(name="candp", bufs=3) as candp,
            tc.tile_pool(name="qp", bufs=1) as qp,
            tc.tile_pool(name="outp", bufs=1) as outp,
            tc.tile_pool(name="ps", bufs=2, space="PSUM") as ps,
        ):
            q_sb = qp.tile([128, _NQ], BF16)
            nc.sync.dma_start(out=q_sb[:, 0:128], in_=q[:, 0:128])
            nc.sync.dma_start(out=q_sb[:, 128:], in_=q[:, 128:])
            tq_sb = qp.tile([128, _NQT], F32)
            nc.sync.dma_start(out=tq_sb[:], in_=tq[:])
            out_sb = outp.tile([128, _NQT * ngrp * 8], F32)
            sums_sb = outp.tile([128, _NQT * ngrp], F32)
            nc.gpsimd.memset(out_sb[:], 0.0)
            nc.gpsimd.memset(sums_sb[:], 0.0)

            cw = sgg * _GRP // 2  # packed cols per supergroup
            j = 0  # scan-tile counter in execution order
            for sg in range(nsg):
                ct = candp.tile([128, cw], BF16, tag="cand")
                nc.sync.dma_start(out=ct[:], in_=cand[:, sg * cw : (sg + 1) * cw])
                for qt in range(_NQT):
                    qa = q_sb[0:64, qt * 128 : (qt + 1) * 128]
                    qb = q_sb[64:128, qt * 128 : (qt + 1) * 128]
                    for gg in range(sgg):
                        g = sg * sgg + gg
                        pt = ps.tile([128, _GRP], F32, tag="pt")
                        for sb in range(_GRP // 1024):  # superblocks in group
                            c = (gg * 2 + sb) * 512
                            nc.tensor.matmul(
                                pt[:, sb * 1024 : sb * 1024 + 512],
                                qa,
                                ct[0:64, c : c + 512],
                                start=True,
                                stop=True,
                                tile_position=(0, 0),
                            )
                            nc.tensor.matmul(
                                pt[:, sb * 1024 + 512 : sb * 1024 + 1024],
                                qb,
                                ct[64:128, c : c + 512],
                                start=True,
                                stop=True,
                                tile_position=(64, 0),
                            )
                        col = qt * ngrp + g
                        if _is_dve(j):
                            nc.vector.max(
                                out=out_sb[:, col * 8 : (col + 1) * 8],
                                in_=pt[:],
                            )
                        else:
                            nc.scalar.activation(
                                pt[:],
                                pt[:],
                                mybir.ActivationFunctionType.Relu,
                                bias=tq_sb[:, qt : qt + 1],
                                accum_out=sums_sb[:, col : col + 1],
                            )
                        j += 1
            nc.sync.dma_start(out=out[:], in_=out_sb[:])
            nc.sync.dma_start(out=sums[:], in_=sums_sb[:])
    _split_sync_waits(nc)
    return nc


_nc_cache = [None]


def _get_nc():
    if _nc_cache[0] is None:
        _nc_cache[0] = _build()
    return _nc_cache[0]


def _pack_cands(shard_bf16):
    """[n, 64] bf16 -> [128, n//2]: superblocks of 1024 split into two
    512-candidate halves on partition rows [0,64) and [64,128)."""
    n = shard_bf16.shape[0]
    npair = n // 1024
    r = shard_bf16.reshape(npair, 2, 512, _D)  # [pair, half, j, d]
    return np.ascontiguousarray(np.transpose(r, (1, 3, 0, 2)).reshape(128, n // 2))


_last_profile = {}


def kernel(query_embeddings, candidates):
    query_embeddings = np.asarray(query_embeddings, dtype=np.float32)
    candidates = np.asarray(candidates, dtype=np.float32)
    assert query_embeddings.shape == (_NQ, _D)
    assert candidates.shape == (_NCAND, _D)

    # Per-query screening threshold from an exact 1/64 sample pass: the
    # m-th best of the sample sits near global rank 64*m and is below the
    # true 100th-best w.h.p.; rare misses only cost tiny tail-value error.
    sample = np.ascontiguousarray(candidates[::64])
    ss = query_embeddings @ sample.T  # [1024, 16384]
    t_q = (
        -np.partition(-ss, _M_SAMPLE - 1, axis=1)[:, _M_SAMPLE - 1] - _T_MARGIN
    ).astype(np.float32)

    nc = _get_nc()
    import ml_dtypes

    qT = query_embeddings.T.astype(ml_dtypes.bfloat16)  # [64, 1024]
    qfull = np.ascontiguousarray(np.concatenate([qT, qT], axis=0))  # [128, 1024]
    cand_bf16 = candidates.astype(ml_dtypes.bfloat16)
    tq_packed = np.ascontiguousarray(
        (-t_q).reshape(_NQT, 128).T.astype(np.float32)
    )  # [128, 8]
    in_maps = []
    for c in range(_NCORES):
        in_maps.append(
            {
                "q": qfull,
                "cand": _pack_cands(cand_bf16[c * _SHARD : (c + 1) * _SHARD]),
                "tq": tq_packed,
            }
        )
    res = run_bass_kernel_spmd(
        nc, in_maps, core_ids=list(range(_NCORES)), trace=TRACE
    )
    _last_profile["exec_time_ns"] = res.exec_time_ns
    _last_profile["res"] = res

    # Unpack DVE survivors and ACT block screen-sums
    surv, sums = [], []
    for c in range(_NCORES):
        o = res.results[c]["out"]  # [128, NQT*NGRP*8]
        o = (
            o.reshape(128, _NQT, _NGRP * 8)
            .transpose(1, 0, 2)
            .reshape(_NQ, _NGRP * 8)
        )
        surv.append(o)
        sm = res.results[c]["sums"]  # [128, NQT*NGRP]
        sm = sm.reshape(128, _NQT, _NGRP).transpose(1, 0, 2).reshape(_NQ, _NGRP)
        sums.append(sm)
    allsurv = np.concatenate(surv, axis=1)  # [1024, 8*NGRP*8]; ACT cols are 0

    # Host rescore of ACT-flagged blocks (exact fp32 values)
    extras = np.full((_NQ, 1024), -np.inf, dtype=np.float32)
    cnt = np.zeros(_NQ, dtype=np.int64)
    rth = (t_q - 0.05).astype(np.float32)
    for c in range(_NCORES):
        sm = sums[c]  # [1024, NGRP]
        for b in range(_NGRP):
            qs = np.nonzero(sm[:, b] > _SUM_EPS)[0]
            if qs.size == 0:
                continue
            blk = candidates[
                c * _SHARD + b * _GRP : c * _SHARD + (b + 1) * _GRP
            ]  # [GRP, 64]
            sc = query_embeddings[qs] @ blk.T  # [nq, GRP]
            mask = sc > rth[qs, None]
            qh, ch = np.nonzero(mask)
            if qh.size == 0:
                continue
            qg = qs[qh]  # sorted by qh
            vals = sc[qh, ch]
            ranks = np.arange(qg.size) - np.searchsorted(qg, qg, side="left")
            pos = np.minimum(cnt[qg] + ranks, extras.shape[1] - 1)
            extras[qg, pos] = np.maximum(extras[qg, pos], vals)
            np.add.at(cnt, qg, 1)
    pool = np.concatenate([allsurv, extras], axis=1)

    # Exact top-100 merge
    part = np.partition(pool, pool.shape[1] - _K, axis=1)[:, -_K:]
    top = -np.sort(-part, axis=1)
    return top.astype(np.float32)



# revision 2
# speedup vs baseline: 1.1646x; 1.1646x over previous
"""DatasetTopK Trainium2 kernel.

Problem: query_embeddings [1024, 64] f32, candidates [1048576, 64] f32
-> per-query top-100 scores (sorted desc), scores = Q @ C^T.

Strategy (8 NeuronCores, candidates sharded 131072/core):
  - Host: transpose + pack each core's candidate shard into [128, 65536]
    (superblocks of 1024 candidates split across the two 64-partition
    halves, enabling 2-way row-tiled K=64 matmuls on the PE array).
  - Device: bf16 matmuls (full PE rate, ~0.1 abs err) -> PSUM. The scan
    runs at 1024-element granularity over FOUR rotating 2-bank PSUM
    slots so PE refill of slot k overlaps scans of slots k+1..k+3
    (the only two PSUM-capable engines stay ~100% busy):
      * even 1024-blocks: DVE max8 (exact top-8 of the block)
      * odd  1024-blocks: ACT relu(s - t_q) + accum (block screening);
        host rescores flagged blocks exactly.
  - Host: thresholds t_q from an exact 1/64 sample pass; final exact
    top-100 merge of DVE survivors + rescored ACT candidates.
"""

import numpy as np

import concourse.bass as bass
import concourse.mybir as mybir
from concourse.tile import TileContext
from concourse.bass_utils import run_bass_kernel_spmd

F32 = mybir.dt.float32
BF16 = mybir.dt.bfloat16

_NCORES = 8
_NQ = 1024
_D = 64
_NCAND = 1048576
_SHARD = _NCAND // _NCORES  # 131072
_GRP = 1024  # candidates per scan tile = 2 PSUM banks
_NGRP = _SHARD // _GRP  # 128 blocks per core
_SGG = 8  # blocks per supergroup (DMA chunk: 8192 cands = 1 MiB bf16)
_NSG = _NGRP // _SGG  # 16 supergroups
_NQT = 8  # query tiles of 128
_K = 100

_SUM_EPS = 0.01  # ACT screen sum > eps -> host rescore
_M_SAMPLE = 6  # threshold = m-th best of the 1/64 sample
_T_MARGIN = 0.10

TRACE = False  # set by test harness for profiling runs

_ctr = [0]


def _split_sync_waits(nc, max_waits=1):
    """Workaround for walrus 'Too many sync wait commands': move excess
    per-instruction sync-waits onto preceding same-engine NOPs."""
    for f in nc.m.functions:
        for b in f.blocks:
            new_insts = []
            changed = False
            for ins in b.instructions:
                si = ins.sync_info
                if si is not None and len(si.on_wait) > max_waits:
                    waits = list(si.on_wait)
                    head, rest = waits[: -max_waits], waits[-max_waits:]
                    for i in range(0, len(head), max_waits):
                        _ctr[0] += 1
                        nop = mybir.InstNoOp(
                            name=f"I-waitsplit-{_ctr[0]}",
                            engine=ins.engine,
                            sync_info=mybir.SyncInfo(
                                on_wait=head[i : i + max_waits], on_update=[]
                            ),
                        )
                        nc.register_instruction(nop, overwrite=True)
                        new_insts.append(nop)
                        changed = True
                    ins.sync_info = mybir.SyncInfo(
                        on_wait=rest, on_update=list(si.on_update)
                    )
                new_insts.append(ins)
            if changed:
                b.instructions = new_insts
    return nc


def _build(nsg=_NSG, sgg=_SGG):
    ngrp = nsg * sgg
    shard = ngrp * _GRP
    ndve = ngrp // 2  # even blocks -> DVE, odd -> ACT
    nc = bass.Bass()
    q = nc.dram_tensor("q", [128, _NQ], BF16, kind="ExternalInput")
    cand = nc.dram_tensor("cand", [128, shard // 2], BF16, kind="ExternalInput")
    tq = nc.dram_tensor("tq", [128, _NQT], F32, kind="ExternalInput")
    # out col = (g//2)*NQT + qt (g-major for per-supergroup DMA out)
    out = nc.dram_tensor("out", [128, ndve * _NQT * 8], F32, kind="ExternalOutput")
    sums = nc.dram_tensor("sums", [128, ndve * _NQT], F32, kind="ExternalOutput")

    with TileContext(nc) as tc:
        with (
            tc.tile_pool(name="candp", bufs=3) as candp,
            tc.tile_pool(name="qp", bufs=1) as qp,
            tc.tile_pool(name="outp", bufs=1) as outp,
            tc.tile_pool(name="ps", bufs=4, space="PSUM") as ps,
        ):
            q_sb = qp.tile([128, _NQ], BF16)
            nc.sync.dma_start(out=q_sb[:, 0:128], in_=q[:, 0:128])
            nc.sync.dma_start(out=q_sb[:, 128:], in_=q[:, 128:])
            tq_sb = qp.tile([128, _NQT], F32)
            nc.sync.dma_start(out=tq_sb[:], in_=tq[:])
            out_sb = outp.tile([128, ndve * _NQT * 8], F32)
            sums_sb = outp.tile([128, ndve * _NQT], F32)

            cw = sgg * _GRP // 2  # packed cols per supergroup (4096)
            for sg in range(nsg):
                ct = candp.tile([128, cw], BF16, tag="cand")
                nc.sync.dma_start(out=ct[:], in_=cand[:, sg * cw : (sg + 1) * cw])
                for qt in range(_NQT):
                    qa = q_sb[0:64, qt * 128 : (qt + 1) * 128]
                    qb = q_sb[64:128, qt * 128 : (qt + 1) * 128]
                    for blk in range(sgg):
                        g = sg * sgg + blk
                        pt = ps.tile([128, _GRP], F32, tag="pt")
                        c = blk * 512
                        nc.tensor.matmul(
                            pt[:, 0:512],
                            qa,
                            ct[0:64, c : c + 512],
                            start=True,
                            stop=True,
                            tile_position=(0, 0),
                        )
                        nc.tensor.matmul(
                            pt[:, 512:1024],
                            qb,
                            ct[64:128, c : c + 512],
                            start=True,
                            stop=True,
                            tile_position=(64, 0),
                        )
                        col = (g // 2) * _NQT + qt
                        if g % 2 == 0:
                            nc.vector.max(
                                out=out_sb[:, col * 8 : (col + 1) * 8],
                                in_=pt[:],
                            )
                        else:
                            nc.scalar.activation(
                                pt[:],
                                pt[:],
                                mybir.ActivationFunctionType.Relu,
                                bias=tq_sb[:, qt : qt + 1],
                                accum_out=sums_sb[:, col : col + 1],
                            )
                # stream this supergroup's finished outputs back to HBM
                o0 = (sg * sgg // 2) * _NQT * 8
                o1 = ((sg + 1) * sgg // 2) * _NQT * 8
                nc.sync.dma_start(out=out[:, o0:o1], in_=out_sb[:, o0:o1])
                s0 = (sg * sgg // 2) * _NQT
                s1 = ((sg + 1) * sgg // 2) * _NQT
                nc.sync.dma_start(out=sums[:, s0:s1], in_=sums_sb[:, s0:s1])
    _split_sync_waits(nc)
    return nc


_nc_cache = [None]


def _get_nc():
    if _nc_cache[0] is None:
        _nc_cache[0] = _build()
    return _nc_cache[0]


def _pack_cands(shard_bf16):
    """[n, 64] bf16 -> [128, n//2]: superblocks of 1024 split into two
    512-candidate halves on partition rows [0,64) and [64,128)."""
    n = shard_bf16.shape[0]
    npair = n // 1024
    r = shard_bf16.reshape(npair, 2, 512, _D)  # [pair, half, j, d]
    return np.ascontiguousarray(np.transpose(r, (1, 3, 0, 2)).reshape(128, n // 2))


_last_profile = {}


def kernel(query_embeddings, candidates):
    query_embeddings = np.asarray(query_embeddings, dtype=np.float32)
    candidates = np.asarray(candidates, dtype=np.float32)
    assert query_embeddings.shape == (_NQ, _D)
    assert candidates.shape == (_NCAND, _D)

    # Per-query screening threshold from an exact 1/64 sample pass: the
    # m-th best of the sample sits near global rank 64*m and is below the
    # true 100th-best w.h.p.; rare misses only cost tiny tail-value error.
    sample = np.ascontiguousarray(candidates[::64])
    ss = query_embeddings @ sample.T  # [1024, 16384]
    t_q = (
        -np.partition(-ss, _M_SAMPLE - 1, axis=1)[:, _M_SAMPLE - 1] - _T_MARGIN
    ).astype(np.float32)

    nc = _get_nc()
    import ml_dtypes

    qT = query_embeddings.T.astype(ml_dtypes.bfloat16)  # [64, 1024]
    qfull = np.ascontiguousarray(np.concatenate([qT, qT], axis=0))  # [128, 1024]
    cand_bf16 = candidates.astype(ml_dtypes.bfloat16)
    tq_packed = np.ascontiguousarray(
        (-t_q).reshape(_NQT, 128).T.astype(np.float32)
    )  # [128, 8]
    in_maps = []
    for c in range(_NCORES):
        in_maps.append(
            {
                "q": qfull,
                "cand": _pack_cands(cand_bf16[c * _SHARD : (c + 1) * _SHARD]),
                "tq": tq_packed,
            }
        )
    res = run_bass_kernel_spmd(
        nc, in_maps, core_ids=list(range(_NCORES)), trace=TRACE
    )
    _last_profile["exec_time_ns"] = res.exec_time_ns
    _last_profile["res"] = res

    # Unpack DVE survivors (even blocks) and ACT screen-sums (odd blocks)
    ndve = _NGRP // 2
    surv, sums = [], []
    for c in range(_NCORES):
        o = res.results[c]["out"]  # [128, ndve*NQT*8], col=(g//2)*NQT+qt
        o = (
            o.reshape(128, ndve, _NQT, 8)
            .transpose(2, 0, 1, 3)
            .reshape(_NQ, ndve * 8)
        )
        surv.append(o)
        sm = res.results[c]["sums"]  # [128, ndve*NQT], col=(g//2)*NQT+qt
        sm = sm.reshape(128, ndve, _NQT).transpose(2, 0, 1).reshape(_NQ, ndve)
        sums.append(sm)
    allsurv = np.concatenate(surv, axis=1)  # [1024, 8*ndve*8]

    # Host rescore of ACT-flagged odd blocks (exact fp32 values)
    extras = np.full((_NQ, 1024), -np.inf, dtype=np.float32)
    cnt = np.zeros(_NQ, dtype=np.int64)
    rth = (t_q - 0.05).astype(np.float32)
    for c in range(_NCORES):
        sm = sums[c]  # [1024, ndve]
        for b2 in range(ndve):
            g = 2 * b2 + 1  # odd blocks are ACT-screened
            qs = np.nonzero(sm[:, b2] > _SUM_EPS)[0]
            if qs.size == 0:
                continue
            blk = candidates[
                c * _SHARD + g * _GRP : c * _SHARD + (g + 1) * _GRP
            ]  # [GRP, 64]
            sc = query_embeddings[qs] @ blk.T  # [nq, GRP]
            mask = sc > rth[qs, None]
            qh, ch = np.nonzero(mask)
            if qh.size == 0:
                continue
            qg = qs[qh]  # sorted by qh
            vals = sc[qh, ch]
            ranks = np.arange(qg.size) - np.searchsorted(qg, qg, side="left")
            pos = np.minimum(cnt[qg] + ranks, extras.shape[1] - 1)
            extras[qg, pos] = np.maximum(extras[qg, pos], vals)
            np.add.at(cnt, qg, 1)
    pool = np.concatenate([allsurv, extras], axis=1)

    # Exact top-100 merge
    part = np.partition(pool, pool.shape[1] - _K, axis=1)[:, -_K:]
    top = -np.sort(-part, axis=1)
    return top.astype(np.float32)


# revision 5
# speedup vs baseline: 1.3705x; 1.1768x over previous
"""DatasetTopK Trainium2 kernel.

Problem: query_embeddings [1024, 64] f32, candidates [1048576, 64] f32
-> per-query top-100 scores (sorted desc), scores = Q @ C^T.

Strategy (8 NeuronCores, candidates sharded 131072/core):
  - Host: transpose + pack each core's candidate shard into [128, 65536]
    (superblocks of 1024 candidates split across the two 64-partition
    halves, enabling 2-way row-tiled K=64 matmuls on the PE array).
  - Device: bf16 matmuls (full PE rate, ~0.1 abs err) -> PSUM f32. The
    scan runs at 1024-element granularity over FOUR rotating 2-bank PSUM
    slots so PE refill of slot k overlaps scans of slots k+1..k+3; the
    only two PSUM-capable engines run ~100% busy, split by measured
    per-tile cost (DVE 1200ns vs ACT 1222ns -> 50.45% DVE):
      * DVE max8: exact top-8 of the 1024-block
      * ACT relu(s - t_q) + accum: block screening; host rescores
        flagged blocks exactly.
  - Host: thresholds t_q from an exact 1/64 sample pass; final exact
    top-100 merge of DVE survivors + rescored ACT candidates.
"""

import numpy as np

import concourse.bass as bass
import concourse.mybir as mybir
from concourse.tile import TileContext
from concourse.bass_utils import run_bass_kernel_spmd

F32 = mybir.dt.float32
BF16 = mybir.dt.bfloat16

_NCORES = 8
_NQ = 1024
_D = 64
_NCAND = 1048576
_SHARD = _NCAND // _NCORES  # 131072
_GRP = 1024  # candidates per scan tile = 2 PSUM banks
_NGRP = _SHARD // _GRP  # 128 blocks per core
_SGG = 8  # blocks per supergroup (DMA chunk: 8192 cands = 1 MiB bf16)
_NSG = _NGRP // _SGG  # 16 supergroups
_NQT = 8  # query tiles of 128
_K = 100
_NTILE = _NQT * _NGRP  # 1024 scan tiles per core

_DVE_FRAC = 0.5045  # measured per-tile cost ratio ACT/(DVE+ACT)

_SUM_EPS = 0.01  # ACT screen sum > eps -> host rescore
_M_SAMPLE = 6  # threshold = m-th best of the 1/64 sample
_T_MARGIN = 0.10

TRACE = False  # set by test harness for profiling runs

_ctr = [0]


def _is_dve(j):
    """Engine for the j-th scan tile (cost-weighted Bresenham mix)."""
    return int((j + 1) * _DVE_FRAC) != int(j * _DVE_FRAC)


def _split_sync_waits(nc, max_waits=1):
    """Workaround for walrus 'Too many sync wait commands': move excess
    per-instruction sync-waits onto preceding same-engine NOPs."""
    for f in nc.m.functions:
        for b in f.blocks:
            new_insts = []
            changed = False
            for ins in b.instructions:
                si = ins.sync_info
                if si is not None and len(si.on_wait) > max_waits:
                    waits = list(si.on_wait)
                    head, rest = waits[: -max_waits], waits[-max_waits:]
                    for i in range(0, len(head), max_waits):
                        _ctr[0] += 1
                        nop = mybir.InstNoOp(
                            name=f"I-waitsplit-{_ctr[0]}",
                            engine=ins.engine,
                            sync_info=mybir.SyncInfo(
                                on_wait=head[i : i + max_waits], on_update=[]
                            ),
                        )
                        nc.register_instruction(nop, overwrite=True)
                        new_insts.append(nop)
                        changed = True
                    ins.sync_info = mybir.SyncInfo(
                        on_wait=rest, on_update=list(si.on_update)
                    )
                new_insts.append(ins)
            if changed:
                b.instructions = new_insts
    return nc


def _build(nsg=_NSG, sgg=_SGG):
    ngrp = nsg * sgg
    shard = ngrp * _GRP
    nc = bass.Bass()
    q = nc.dram_tensor("q", [128, _NQ], BF16, kind="ExternalInput")
    cand = nc.dram_tensor("cand", [128, shard // 2], BF16, kind="ExternalInput")
    tq = nc.dram_tensor("tq", [128, _NQT], F32, kind="ExternalInput")
    # col = g*NQT + qt (g-major for per-supergroup DMA out); host reads
    # only the columns its sink replay says are valid.
    out = nc.dram_tensor("out", [128, ngrp * _NQT * 8], F32, kind="ExternalOutput")
    sums = nc.dram_tensor("sums", [128, ngrp * _NQT], F32, kind="ExternalOutput")

    with TileContext(nc) as tc:
        with (
            tc.tile_pool(name="candp", bufs=3) as candp,
            tc.tile_pool(name="qp", bufs=1) as qp,
            tc.tile_pool(name="outp", bufs=1) as outp,
            tc.tile_pool(name="ps", bufs=4, space="PSUM") as ps,
        ):
            q_sb = qp.tile([128, _NQ], BF16)
            nc.sync.dma_start(out=q_sb[:, 0:128], in_=q[:, 0:128])
            nc.sync.dma_start(out=q_sb[:, 128:], in_=q[:, 128:])
            tq_sb = qp.tile([128, _NQT], F32)
            nc.sync.dma_start(out=tq_sb[:], in_=tq[:])
            out_sb = outp.tile([128, ngrp * _NQT * 8], F32)
            sums_sb = outp.tile([128, ngrp * _NQT], F32)

            cw = sgg * _GRP // 2  # packed cols per supergroup (4096)
            j = 0  # global scan-tile counter
            for sg in range(nsg):
                ct = candp.tile([128, cw], BF16, tag="cand")
                nc.sync.dma_start(out=ct[:], in_=cand[:, sg * cw : (sg + 1) * cw])
                for qt in range(_NQT):
                    qa = q_sb[0:64, qt * 128 : (qt + 1) * 128]
                    qb = q_sb[64:128, qt * 128 : (qt + 1) * 128]
                    for blk in range(sgg):
                        g = sg * sgg + blk
                        pt = ps.tile([128, _GRP], F32, tag="pt")
                        c = blk * 512
                        nc.tensor.matmul(
                            pt[:, 0:512],
                            qa,
                            ct[0:64, c : c + 512],
                            start=True,
                            stop=True,
                            tile_position=(0, 0),
                        )
                        nc.tensor.matmul(
                            pt[:, 512:1024],
                            qb,
                            ct[64:128, c : c + 512],
                            start=True,
                            stop=True,
                            tile_position=(64, 0),
                        )
                        col = g * _NQT + qt
                        if _is_dve(j):
                            nc.vector.max(
                                out=out_sb[:, col * 8 : (col + 1) * 8],
                                in_=pt[:],
                            )
                        else:
                            nc.scalar.activation(
                                pt[:],
                                pt[:],
                                mybir.ActivationFunctionType.Relu,
                                bias=tq_sb[:, qt : qt + 1],
                                accum_out=sums_sb[:, col : col + 1],
                            )
                        j += 1
                # stream this supergroup's finished outputs back to HBM
                o0 = sg * sgg * _NQT * 8
                o1 = (sg + 1) * sgg * _NQT * 8
                nc.sync.dma_start(out=out[:, o0:o1], in_=out_sb[:, o0:o1])
                s0 = sg * sgg * _NQT
                s1 = (sg + 1) * sgg * _NQT
                nc.sync.dma_start(out=sums[:, s0:s1], in_=sums_sb[:, s0:s1])
    _split_sync_waits(nc)
    return nc


_nc_cache = [None]


def _get_nc():
    if _nc_cache[0] is None:
        _nc_cache[0] = _build()
    return _nc_cache[0]


def _pack_cands(shard_bf16):
    """[n, 64] bf16 -> [128, n//2]: superblocks of 1024 split into two
    512-candidate halves on partition rows [0,64) and [64,128)."""
    n = shard_bf16.shape[0]
    npair = n // 1024
    r = shard_bf16.reshape(npair, 2, 512, _D)  # [pair, half, j, d]
    return np.ascontiguousarray(np.transpose(r, (1, 3, 0, 2)).reshape(128, n // 2))


def _tile_info(j):
    """Global tile index -> (qt, g)."""
    sg, rem = divmod(j, _NQT * _SGG)
    qt, blk = divmod(rem, _SGG)
    return qt, sg * _SGG + blk


_last_profile = {}


def kernel(query_embeddings, candidates):
    query_embeddings = np.asarray(query_embeddings, dtype=np.float32)
    candidates = np.asarray(candidates, dtype=np.float32)
    assert query_embeddings.shape == (_NQ, _D)
    assert candidates.shape == (_NCAND, _D)

    # Per-query screening threshold from an exact 1/64 sample pass: the
    # m-th best of the sample sits near global rank 64*m and is below the
    # true 100th-best w.h.p.; rare misses only cost tiny tail-value error.
    sample = np.ascontiguousarray(candidates[::64])
    ss = query_embeddings @ sample.T  # [1024, 16384]
    t_q = (
        -np.partition(-ss, _M_SAMPLE - 1, axis=1)[:, _M_SAMPLE - 1] - _T_MARGIN
    ).astype(np.float32)

    nc = _get_nc()
    import ml_dtypes

    qT = query_embeddings.T.astype(ml_dtypes.bfloat16)  # [64, 1024]
    qfull = np.ascontiguousarray(np.concatenate([qT, qT], axis=0))  # [128, 1024]
    cand_bf16 = candidates.astype(ml_dtypes.bfloat16)
    tq_packed = np.ascontiguousarray(
        (-t_q).reshape(_NQT, 128).T.astype(np.float32)
    )  # [128, 8]
    in_maps = []
    for c in range(_NCORES):
        in_maps.append(
            {
                "q": qfull,
                "cand": _pack_cands(cand_bf16[c * _SHARD : (c + 1) * _SHARD]),
                "tq": tq_packed,
            }
        )
    res = run_bass_kernel_spmd(
        nc, in_maps, core_ids=list(range(_NCORES)), trace=TRACE
    )
    _last_profile["exec_time_ns"] = res.exec_time_ns
    _last_profile["res"] = res

    # Tile classification (same on every core)
    dmap = [[] for _ in range(_NQT)]
    amap = {}
    for j in range(_NTILE):
        qt, g = _tile_info(j)
        if _is_dve(j):
            dmap[qt].append(g)
        else:
            amap.setdefault(g, []).append(qt)
    nsurv = max(len(dmap[qt]) for qt in range(_NQT)) * 8

    # Per-query survivor pool from DVE block top-8s
    surv_parts = []
    sums = []
    for c in range(_NCORES):
        o = res.results[c]["out"]  # [128, NGRP*NQT*8], col = g*NQT+qt
        o = o.reshape(128, _NGRP, _NQT, 8)
        sv = np.full((_NQ, nsurv), -np.inf, dtype=np.float32)
        for qt in range(_NQT):
            dv = o[:, dmap[qt], qt, :].reshape(128, -1)
            sv[qt * 128 : (qt + 1) * 128, : dv.shape[1]] = dv
        surv_parts.append(sv)
        sums.append(res.results[c]["sums"].reshape(128, _NGRP, _NQT))
    allsurv = np.concatenate(surv_parts, axis=1)

    # Host rescore of ACT-flagged blocks (exact fp32 values)
    extras = np.full((_NQ, 1024), -np.inf, dtype=np.float32)
    cnt = np.zeros(_NQ, dtype=np.int64)
    rth = (t_q - 0.05).astype(np.float32)
    for c in range(_NCORES):
        sm = sums[c]  # [128, NGRP, NQT]
        for g, qts in amap.items():
            qlist = []
            for qt in qts:
                part = np.nonzero(sm[:, g, qt] > _SUM_EPS)[0]
                if part.size:
                    qlist.append(qt * 128 + part)
            if not qlist:
                continue
            qs = np.sort(np.concatenate(qlist))
            blk = candidates[
                c * _SHARD + g * _GRP : c * _SHARD + (g + 1) * _GRP
            ]  # [GRP, 64]
            sc = query_embeddings[qs] @ blk.T  # [nq, GRP]
            mask = sc > rth[qs, None]
            qh, ch = np.nonzero(mask)
            if qh.size == 0:
                continue
            qg = qs[qh]  # sorted by qh
            vals = sc[qh, ch]
            ranks = np.arange(qg.size) - np.searchsorted(qg, qg, side="left")
            pos = np.minimum(cnt[qg] + ranks, extras.shape[1] - 1)
            extras[qg, pos] = np.maximum(extras[qg, pos], vals)
            np.add.at(cnt, qg, 1)
    pool = np.concatenate([allsurv, extras], axis=1)

    # Exact top-100 merge
    part = np.partition(pool, pool.shape[1] - _K, axis=1)[:, -_K:]
    top = -np.sort(-part, axis=1)
    return top.astype(np.float32)
